# revision 48
# baseline (speedup 1.0000x reference)
"""Bass/Trainium2 kernel for DynamicMultiheadAttention (sparse_attention).

Sharding: 8 cores = (batch b in {0,1}) x (query-slice of 512 rows).
Each core computes all 8 heads for its (b, n-slice) in transposed
orientation: scores sT[m, n] with keys m on partitions.

The relative-mask bias is applied multiplicatively after the exp:
  exp(s + rel) = exp(s) * E,  E[h,m,n] = exp(-sum_r c[h,r]*attn_mask)
(shifted by the softmax-invariant constant sum_r c[h,r] so E is in
(0,1]). E is precomputed on the host as bf16 planes; the all-16-bit
tensor_tensor multiply runs in the DVE 2x_1p fast mode (~593ns per
two-head tile). This removes all mask identity matmuls (previously
~82us, 40% of PE time) from the PE critical path.

Fully-padded key tiles (all 128 keys masked by key_padding_mask) are
skipped at program-build time; partially padded tiles are handled by
zeroing the affected rows of V and of the appended ones-column.
The k bias is softmax-invariant and dropped; the v bias folds into the
output bias (softmax rows sum to 1): bo' = bv @ Wo + bo.

Every TPB instruction encoding in this walrus build tolerates only ONE
semaphore wait; a post-pass (_split_matmul_waits) moves extra waits onto
standalone single-wait EventSemaphore instructions inserted before the
offending instruction on the same engine queue.
"""

import numpy as np
import ml_dtypes
import os

def _B(name, default):
    return int(os.environ.get("KB_" + name, default))

N, B, D = 2048, 2, 512
H, R = 8, 3
C = D // H          # 64
NS = N // 4         # 512 query rows per core
NCORES = 8
MT = N // 128       # 16 key tiles

_cache = {}


def _build_program(active, reps=1):
    import concourse.bass as bass
    import concourse.mybir as mybir
    import concourse.tile as tile
    from contextlib import ExitStack

    f32 = mybir.dt.float32
    f32r = mybir.dt.float32r
    bf16 = mybir.dt.bfloat16
    u8 = mybir.dt.uint8
    AFT = mybir.ActivationFunctionType
    ALU = mybir.AluOpType

    MTA = len(active)

    nc = bass.Bass()

    xtq = nc.declare_dram_parameter("xtq", [D, NS], bf16, isOutput=False)
    xtk = nc.declare_dram_parameter("xtk", [D, N], bf16, isOutput=False)
    xtv = nc.declare_dram_parameter("xtv", [D, N], bf16, isOutput=False)
    # E planes, partition-major so one DMA chunk is contiguous per partition:
    # [hp, p(=m%128), mi, j(head in pair), n]. bf16 (not u8): the all-16-bit
    # tensor_tensor E-multiply runs in the DVE 2x_1p fast mode.
    epl = nc.declare_dram_parameter("epl", [H // 2, 128, MTA, 2, NS], bf16,
                                    isOutput=False)
    wq = nc.declare_dram_parameter("wq", [D, D], bf16, isOutput=False)
    wk = nc.declare_dram_parameter("wk", [D, D], bf16, isOutput=False)
    wv = nc.declare_dram_parameter("wv", [D, D], bf16, isOutput=False)
    wo = nc.declare_dram_parameter("wo", [D, D], bf16, isOutput=False)
    bq2 = nc.declare_dram_parameter("bq2", [128, 4], f32, isOutput=False)
    bo2 = nc.declare_dram_parameter("bo2", [128, 4], f32, isOutput=False)
    # per-active-tile pad multiplier planes (zero padded key rows of V)
    pad = nc.declare_dram_parameter("pad", [128, MTA], f32, isOutput=False)
    pad8 = nc.declare_dram_parameter("pad8", [128, MTA, H], f32, isOutput=False)
    outT = nc.declare_dram_parameter("outT", [D, NS], f32, isOutput=True)

    with tile.TileContext(nc) as tc, ExitStack() as ctx:
        mm = nc.tensor.matmul
        _run_once(nc, tc, ctx, mm, tile, mybir, f32, f32r, bf16, u8,
                  AFT, ALU, active, xtq, xtk, xtv, epl, wq, wk, wv, wo,
                  bq2, bo2, pad, pad8, outT)

    _split_matmul_waits(nc, mybir)
    return nc


def _run_once(nc, tc, ctx, mm, tile, mybir, f32, f32r, bf16, u8, AFT, ALU,
              active, xtq, xtk, xtv, epl, wq, wk, wv, wo, bq2, bo2,
              pad, pad8, outT):
    from contextlib import ExitStack
    MTA = len(active)
    with ExitStack() as ctx:
        const_pool = ctx.enter_context(tc.tile_pool(name="const", bufs=1))
        persist = ctx.enter_context(tc.tile_pool(name="persist", bufs=1))

        # constants ride the Pool queue: SP's 650ns-per-DMA dispatch rate is
        # the lead-in bottleneck, so it is reserved for the q/k-path inputs
        loads = []
        bq_sb = const_pool.tile([128, 4], f32)
        loads.append(nc.gpsimd.dma_start(bq_sb[:], bq2[:]))
        bo_sb = const_pool.tile([128, 4], f32)
        loads.append(nc.gpsimd.dma_start(bo_sb[:], bo2[:]))
        pad_sb = const_pool.tile([128, MTA], f32)
        loads.append(nc.gpsimd.dma_start(pad_sb[:], pad[:]))
        pad8_sb = const_pool.tile([128, MTA, H], f32)
        loads.append(nc.gpsimd.dma_start(pad8_sb[:], pad8[:]))
        ones_sb = const_pool.tile([1, 64], f32)
        loads.append(nc.vector.memset(ones_sb[:], 1.0))
        wo_sb = persist.tile([128, 4, D], bf16)

        kT_sb = persist.tile([128, 4, N], bf16)
        qT_sb = persist.tile([128, 4, NS], bf16)
        v_sb = persist.tile([128, MTA, H, C + 1], bf16)
        OT_sb = persist.tile([128, 4, NS], bf16)
        outT_sb = persist.tile([128, 4, NS], f32)

        # ---- Phase A (part 1): DMAs + projections needed by pass 0 ----
        # Emission order software-pipelines the phases: q/k projections for
        # heads 0-3 (j-blocks 0,1) and all of v before pass 0; j-blocks 2,3
        # and the wo load are deferred into pass 0's normalize window.
        xw_pool = ctx.enter_context(tc.tile_pool(name="xw", bufs=1))
        wq_sb = xw_pool.tile([128, 4, D], bf16, tag="w")
        wk_sb = xw_pool.tile([128, 4, D], bf16, tag="w2")
        wv_sb = xw_pool.tile([128, 4, D], bf16, tag="w3")
        xtq_sb = xw_pool.tile([128, 4, NS], bf16, tag="xq")
        xtk_sb = xw_pool.tile([128, 4, N], bf16, tag="xk")
        xtv_sb = xw_pool.tile([128, 4, N], bf16, tag="xv")
        # q/k path on SP, v path on the Pool queue: parallel dispatch halves
        # the dispatch-bound lead-in. One consolidated DMA per weight matrix;
        # x_k / x_v arrive in column chunks so the first projection blocks
        # start before the full 2MB lands.
        nc.sync.dma_start(wq_sb[:], wq[:].rearrange("(c p) d -> p c d", p=128))
        nc.sync.dma_start(xtq_sb[:],
                          xtq[:].rearrange("(c p) n -> p c n", p=128))
        nc.sync.dma_start(wk_sb[:], wk[:].rearrange("(c p) d -> p c d", p=128))
        nc.gpsimd.dma_start(wv_sb[:],
                            wv[:].rearrange("(c p) d -> p c d", p=128))
        for mb in range(4):
            sl = slice(mb * 512, (mb + 1) * 512)
            nc.sync.dma_start(
                xtk_sb[:, :, sl],
                xtk[:, sl].rearrange("(kc p) m -> p kc m", p=128))
            nc.gpsimd.dma_start(
                xtv_sb[:, :, sl],
                xtv[:, sl].rearrange("(kc p) m -> p kc m", p=128))

        # E-plane tiles: quarter planes [128, 4, 2, NS] bf16 per fetch; the
        # first two ride the SP queue right after the input DMAs (so they
        # don't preempt them on the DMA engines), the rest go on the Pool
        # queue spread through the passes
        ep_pool = ctx.enter_context(tc.tile_pool(name="ep", bufs=_B("EP", 6)))
        pT_pool = ctx.enter_context(tc.tile_pool(name="pT", bufs=_B("PT", 5)))
        p2_pool = ctx.enter_context(tc.tile_pool(name="p2", bufs=_B("P2", 5)))
        small_pool = ctx.enter_context(tc.tile_pool(name="small", bufs=8))
        EH = 4
        nequarters = (MTA + EH - 1) // EH
        ep_tiles = {}

        def fetch_e(hp, q, eng=None):
            h0 = q * EH
            hn = min(EH, MTA - h0)
            t = ep_pool.tile([128, EH, 2, NS], bf16, tag="ep")
            (eng or nc.gpsimd).dma_start(t[:, 0:hn, :, :],
                                         epl[hp, :, h0:h0 + hn, :, :])
            ep_tiles[(hp, q)] = t

        # phase-A E fetches ride the Pool queue AFTER the v-path inputs: the
        # DMA device serves queue heads first-come, so this ordering keeps
        # the projection inputs ahead of the (less urgent) E planes. ACT is
        # unusable for this: its DMA dispatch costs ~4.4us per copy and
        # would stall the first evacuation activations.
        fetch_e(0, 0)
        fetch_e(1, 0)
        fetch_e(0, 1)
        fetch_e(1, 1)
        fetch_e(0, 2)
        fetch_e(1, 2)

        vones = [nc.vector.tensor_copy(
            v_sb[:, :, :, C : C + 1],
            pad8_sb[:, :, :].rearrange("p m (h o) -> p m h o", o=1))]

        with tc.tile_pool(name="psA", bufs=_B("PSA", 8), space="PSUM") as psA:
            # qT[dh, n] = (Wq/8).T @ xT_q  (+ bq/8 per-partition), heads 0-3
            for j in range(2):
                ps = psA.tile([128, NS], f32, tag="psA")
                for kc in range(4):
                    mm(ps[:], wq_sb[:, kc, j * 128:(j + 1) * 128],
                       xtq_sb[:, kc, :], start=(kc == 0), stop=(kc == 3))
                nc.scalar.activation(qT_sb[:, j, :], ps[:], AFT.Identity,
                                     bias=bq_sb[:, j:j + 1])

            # kT[dh, m] = Wk.T @ xT_k, heads 0-3 (k bias drops in softmax)
            for mb in range(4):
                for j in range(2):
                    ps = psA.tile([128, NS], f32, tag="psA")
                    for kc in range(4):
                        mm(ps[:], wk_sb[:, kc, j * 128:(j + 1) * 128],
                           xtk_sb[:, kc, mb * 512:(mb + 1) * 512],
                           start=(kc == 0), stop=(kc == 3))
                    nc.scalar.copy(kT_sb[:, j, mb * 512:(mb + 1) * 512], ps[:])

            # v[m, c] = xT_v.T @ Wv, padded key rows zeroed (scale by pad01)
            for mi, mt in enumerate(active):
                ps = psA.tile([128, D], f32, tag="psA")
                for kc in range(4):
                    mm(ps[:], xtv_sb[:, kc, mt * 128:(mt + 1) * 128],
                       wv_sb[:, kc, :], start=(kc == 0), stop=(kc == 3))
                nc.vector.tensor_scalar(
                    v_sb[:, mi, :, 0:C],
                    ps[:].rearrange("p (h c) -> p h c", h=H),
                    pad_sb[:, mi:mi + 1], None, ALU.mult)

        # PSUM pools for phase B (psA released its banks above)
        # psS tiles are [128, 2, NS] (2 banks): two heads' scores per tile so
        # one Exp instruction covers 1024 elements. 2 bufs (4 banks) + psO 4.
        psO = ctx.enter_context(tc.tile_pool(name="psO", bufs=4, space="PSUM"))
        psS = ctx.enter_context(tc.tile_pool(name="psS", bufs=_B("PSS", 2), space="PSUM"))

        # ---- Phase B: attention, two passes of 4 heads (2 head pairs) ----
        def attn_pass(p):
            o_ps = [psO.tile([128, NS], f32, tag="psO", name=f"o_ps{p}_{i}")
                    for i in range(4)]
            for mi in range(MTA):
                for hpl in range(2):
                    hp = 2 * p + hpl
                    if p == 0 and hpl == 0 and mi in (6, 8, 10, 12):
                        nq = {6: [(0, 3), (1, 3)], 8: [(2, 0), (3, 0)],
                              10: [(2, 1), (3, 1)], 12: [(2, 2), (3, 2)]}[mi]
                        for a, b in nq:
                            fetch_e(a, b)
                    if p == 1 and hpl == 0 and mi == 4:
                        fetch_e(2, 3)
                        fetch_e(3, 3)
                    s_ps = psS.tile([128, 2, NS], f32, tag="psS")
                    for j in range(2):
                        h = 4 * p + 2 * hpl + j
                        hj, ho = h // 2, (h % 2) * 64
                        mm(s_ps[:, j, :],
                           kT_sb[ho:ho + 64, hj, active[mi] * 128:active[mi] * 128 + 128],
                           qT_sb[ho:ho + 64, hj, :], start=True, stop=True)
                    pT = pT_pool.tile([128, 2, NS], bf16, tag="pT")
                    nc.scalar.activation(pT[:], s_ps[:], AFT.Exp)
                    p2 = p2_pool.tile([128, 2, NS], bf16, tag="p2")
                    # all-bf16 tensor_tensor: DVE 2x_1p fast mode (~593ns)
                    nc.vector.tensor_tensor(
                        p2[:], pT[:],
                        ep_tiles[(hp, mi // EH)][:, mi % EH, :, :], ALU.mult)
                    for j in range(2):
                        h = 4 * p + 2 * hpl + j
                        mm(o_ps[2 * hpl + j][0:65, :], v_sb[:, mi, h, :],
                           p2[:, j, :],
                           start=(mi == 0), stop=(mi == MTA - 1))
            return o_ps

        def normalize(p, o_ps, halves=(0, 1)):
            # OT[h-rows, n] = o[c, n] / rowsum[n]. All reciprocal broadcasts
            # of this call pack into ONE psS tile (two per bank on opposite
            # partition halves) and are read straight from PSUM, so the psS
            # ring turns over once instead of four times per pass.
            bps = psS.tile([128, 2, NS], f32, tag="psS", name=f"bps{p}")
            for hx, half in enumerate(halves):
                idx = [2 * half, 2 * half + 1]
                rsbs = []
                for i in idx:
                    rsb = small_pool.tile([1, NS], f32, tag="rsb",
                                          name=f"rsb{p}_{i}")
                    nc.vector.reciprocal(rsb[:], o_ps[i][64:65, :])
                    rsbs.append(rsb)
                for k, i in enumerate(idx):
                    mm(bps[64 * k:64 * k + 64, hx, :], ones_sb[0:1, :],
                       rsbs[k][0:1, :], start=True, stop=True)
                b_sb = small_pool.tile([128, NS], f32, tag="bsb",
                                       name=f"bsb{p}_{half}")
                nc.vector.tensor_copy(b_sb[:], bps[:, hx, :])
                for k, i in enumerate(idx):
                    h = 4 * p + i
                    hj, ho = h // 2, (h % 2) * 64
                    nc.vector.tensor_tensor(
                        OT_sb[ho:ho + 64, hj, :], o_ps[i][0:64, :],
                        b_sb[64 * k:64 * k + 64, :], ALU.mult)

        # deferred projections for heads 4-7 (j-blocks 2,3): the first k block
        # goes through the psS ring right at pass-0 end; the rest run in psO
        # slots as pass 0's normalize halves release them, keeping the psS
        # ring free so pass 1's score pipeline restarts immediately
        def proj_j23_k_psS(mb):
            ps = psS.tile([128, 2, NS], f32, tag="psS", name=f"kp{mb}")
            for j in (2, 3):
                for kc in range(4):
                    mm(ps[:, j - 2, :], wk_sb[:, kc, j * 128:(j + 1) * 128],
                       xtk_sb[:, kc, mb * 512:(mb + 1) * 512],
                       start=(kc == 0), stop=(kc == 3))
            for j in (2, 3):
                nc.vector.tensor_copy(
                    kT_sb[:, j, mb * 512:(mb + 1) * 512], ps[:, j - 2, :])

        def proj_j23_q_psS():
            ps = psS.tile([128, 2, NS], f32, tag="psS", name="qp23")
            for j in (2, 3):
                for kc in range(4):
                    mm(ps[:, j - 2, :], wq_sb[:, kc, j * 128:(j + 1) * 128],
                       xtq_sb[:, kc, :], start=(kc == 0), stop=(kc == 3))
            for j in (2, 3):
                nc.scalar.activation(qT_sb[:, j, :], ps[:, j - 2, :],
                                     AFT.Identity, bias=bq_sb[:, j:j + 1])

        def proj_j23_k_psO(mb):
            for j in (2, 3):
                ps = psO.tile([128, NS], f32, tag="psO", name=f"kp{mb}_{j}")
                for kc in range(4):
                    mm(ps[:], wk_sb[:, kc, j * 128:(j + 1) * 128],
                       xtk_sb[:, kc, mb * 512:(mb + 1) * 512],
                       start=(kc == 0), stop=(kc == 3))
                nc.vector.tensor_copy(
                    kT_sb[:, j, mb * 512:(mb + 1) * 512], ps[:])

        o_ps0 = attn_pass(0)
        proj_j23_k_psS(0)
        proj_j23_q_psS()
        normalize(0, o_ps0, (0,))
        proj_j23_k_psO(1)
        normalize(0, o_ps0, (1,))
        proj_j23_k_psO(2)
        proj_j23_k_psO(3)
        for c in range(4):
            nc.sync.dma_start(wo_sb[:, c, :], wo[c * 128:(c + 1) * 128, :])

        o_ps1 = attn_pass(1)

        # ---- Phase C: output projection, pipelined with pass-1 normalize.
        # After half 0 (heads 4,5 -> OT[:,2]) two psO slots free: start the
        # g=0..2 partial accumulation for the first two output blocks; the
        # g=3 term and the remaining blocks follow half 1.
        def outproj_partial(jt):
            ps = psO.tile([128, NS], f32, tag="psO", name=f"oc{jt}")
            for g in (0, 1, 2):
                mm(ps[:], wo_sb[:, g, jt * 128:(jt + 1) * 128],
                   OT_sb[:, g, :], start=(g == 0), stop=False)
            return ps

        def outproj_finish(jt, ps=None):
            if ps is None:
                ps = psO.tile([128, NS], f32, tag="psO", name=f"oc{jt}")
                for g in (0, 1, 2):
                    mm(ps[:], wo_sb[:, g, jt * 128:(jt + 1) * 128],
                       OT_sb[:, g, :], start=(g == 0), stop=False)
            mm(ps[:], wo_sb[:, 3, jt * 128:(jt + 1) * 128],
               OT_sb[:, 3, :], start=False, stop=True)
            nc.scalar.activation(outT_sb[:, jt, :], ps[:], AFT.Identity,
                                 bias=bo_sb[:, jt:jt + 1])
            nc.sync.dma_start(outT[jt * 128:(jt + 1) * 128, :],
                              outT_sb[:, jt, :])

        normalize(1, o_ps1, (0,))
        pc0 = outproj_partial(0)
        pc1 = outproj_partial(1)
        normalize(1, o_ps1, (1,))
        outproj_finish(0, pc0)
        outproj_finish(1, pc1)
        outproj_finish(2)
        outproj_finish(3)


# every TPB instruction encoding in this walrus build tolerates only a
# single semaphore wait -- split extras regardless of opcode
_NO_SPLIT_TYPES = {"InstEventSemaphore"}


def _split_matmul_waits(nc, mybir):
    """Several engine instruction encodings tolerate only one semaphore
    wait; move extra waits onto standalone single-wait EventSemaphore
    instructions inserted right before them on the same engine queue."""
    import bass_rust

    n = 0
    for bb in nc.m.functions[0].blocks:
        insts = list(bb.instructions)
        out = []
        changed = False
        for i in insts:
            si = i.sync_info
            if (type(i).__name__ not in _NO_SPLIT_TYPES and si is not None
                    and len(si.on_wait) > 1):
                w = list(si.on_wait)
                for wx in w[:-1]:
                    ev = mybir.InstEventSemaphore(name=f"mmw_{n}_{i.name}",
                                                  ins=[], outs=[])
                    ev.engine = i.engine
                    ev.sync_info = bass_rust.SyncInfo(on_wait=[wx],
                                                      on_update=[])
                    out.append(ev)
                    n += 1
                si.on_wait = [w[-1]]
                changed = True
            out.append(i)
        if changed:
            bb.instructions = out


def _host_prep(inputs):
    x_q = np.asarray(inputs["x_q"], np.float32)
    x_k = np.asarray(inputs["x_k"], np.float32)
    x_v = np.asarray(inputs["x_v"], np.float32)
    attn_mask = np.asarray(inputs["attn_mask"]).astype(bool)
    kpm = np.asarray(inputs["key_padding_mask"]).astype(bool)
    Wq = np.asarray(inputs["Wq"], np.float32)
    Wk = np.asarray(inputs["Wk"], np.float32)
    Wv = np.asarray(inputs["Wv"], np.float32)
    Wo = np.asarray(inputs["Wo"], np.float32)
    bq = np.asarray(inputs["bq"], np.float32)
    bv = np.asarray(inputs["bv"], np.float32)
    bo = np.asarray(inputs["bo"], np.float32)
    mw = np.asarray(inputs["mask_weight"], np.float64)

    # c[h,r] = softmax(mask_weight[h,:R]) * mask_weight[h,R]
    e = np.exp(mw[:, :R] - mw[:, :R].max(axis=1, keepdims=True))
    w = e / e.sum(axis=1, keepdims=True)
    c = (w * mw[:, R:R + 1]).astype(np.float32)          # [H, R]

    # active key tiles (at least one unpadded key) -- shared across batch
    # so a single compiled program serves all cores
    tile_padded = kpm.reshape(B, MT, 128).all(axis=2)    # [B, MT]
    active = [mt for mt in range(MT) if not tile_padded[:, mt].all()]
    MTA = len(active)

    scale = np.float32(1.0 / np.sqrt(C))
    wq_s = (Wq * scale).astype(np.float32)
    bq_s = (bq * scale).astype(np.float32)
    bo_p = (bv @ Wo + bo).astype(np.float32)

    bq2 = np.ascontiguousarray(bq_s.reshape(4, 128).T)
    bo2 = np.ascontiguousarray(bo_p.reshape(4, 128).T)

    bf = ml_dtypes.bfloat16
    common = dict(wq=wq_s.astype(bf), wk=Wk.astype(bf), wv=Wv.astype(bf),
                  wo=Wo.astype(bf), bq2=bq2, bo2=bo2)

    # E planes per (b, n-slice): E[h, mi, p, n] = round(255*exp(-bias))
    emul = np.exp(-c)                                    # [H, R] in (0,1]
    in_maps = []
    for core in range(NCORES):
        b, ns = core // 4, core % 4
        n0 = ns * NS
        pad01 = (~kpm[b]).astype(np.float32)             # [N]
        pad2 = np.ascontiguousarray(
            pad01.reshape(MT, 128).T[:, active])         # [128, MTA]
        pad8 = np.ascontiguousarray(np.repeat(pad2[:, :, None], H, axis=2))
        # reference adds rel = +sum_r c[h,r]*(~attn_mask); shifting by the
        # softmax-invariant constant sum_r c[h,r] gives the bounded form
        # E = exp(-sum_r c[h,r]*attn_mask) in (0,1], ideal for u8
        inv = attn_mask[b, :, n0:n0 + NS, :]             # [R, NS, N]
        # compute per active tile to keep memory small, u8 quantized
        # partition-major: [hp, p(=m within tile), mi, j(head in pair), n]
        ep = np.empty((H // 2, 128, MTA, 2, NS), bf)
        for mi, mt in enumerate(active):
            invt = inv[:, :, mt * 128:(mt + 1) * 128]    # [R, NS, 128]
            bias = np.einsum('hr,rnm->hmn', c, invt.astype(np.float32))
            ep[:, :, mi] = np.exp(-bias).astype(bf).reshape(
                H // 2, 2, 128, NS).transpose(0, 2, 1, 3)
        ep = np.ascontiguousarray(ep)
        m = dict(common)
        m["xtq"] = np.ascontiguousarray(x_q[n0:n0 + NS, b, :].T).astype(bf)
        m["xtk"] = np.ascontiguousarray(x_k[:, b, :].T).astype(bf)
        m["xtv"] = np.ascontiguousarray(x_v[:, b, :].T).astype(bf)
        m["epl"] = ep
        m["pad"] = pad2
        m["pad8"] = pad8
        in_maps.append(m)
    return in_maps, active


def kernel(**inputs) -> np.ndarray:
    from concourse.bass_utils import run_bass_kernel_spmd

    in_maps, active = _host_prep(inputs)
    key = tuple(active)
    if key not in _cache:
        _cache[key] = _build_program(active)
        _cache["nc"] = _cache[key]
    nc = _cache[key]

    res = run_bass_kernel_spmd(nc, in_maps, list(range(NCORES)))

    out = np.empty((N, B, D), np.float32)
    for core in range(NCORES):
        b, ns = core // 4, core % 4
        n0 = ns * NS
        out[n0:n0 + NS, b, :] = res.results[core]["outT"].T
    return out


# revision 50
# speedup vs baseline: 1.0054x; 1.0054x over previous
"""Bass/Trainium2 kernel for DynamicMultiheadAttention (sparse_attention).

Sharding: 8 cores = (batch b in {0,1}) x (query-slice of 512 rows).
Each core computes all 8 heads for its (b, n-slice) in transposed
orientation: scores sT[m, n] with keys m on partitions.

The relative-mask bias is applied multiplicatively after the exp:
  exp(s + rel) = exp(s) * E,  E[h,m,n] = exp(-sum_r c[h,r]*attn_mask)
(shifted by the softmax-invariant constant sum_r c[h,r] so E is in
(0,1]). E is precomputed on the host as bf16 planes; the all-16-bit
tensor_tensor multiply runs in the DVE 2x_1p fast mode (~593ns per
two-head tile). This removes all mask identity matmuls (previously
~82us, 40% of PE time) from the PE critical path.

Fully-padded key tiles (all 128 keys masked by key_padding_mask) are
skipped at program-build time; partially padded tiles are handled by
zeroing the affected rows of V and of the appended ones-column.
The k bias is softmax-invariant and dropped; the v bias folds into the
output bias (softmax rows sum to 1): bo' = bv @ Wo + bo.

Every TPB instruction encoding in this walrus build tolerates only ONE
semaphore wait; a post-pass (_split_matmul_waits) moves extra waits onto
standalone single-wait EventSemaphore instructions inserted before the
offending instruction on the same engine queue.
"""

import numpy as np
import ml_dtypes
import os

def _B(name, default):
    return int(os.environ.get("KB_" + name, default))

N, B, D = 2048, 2, 512
H, R = 8, 3
C = D // H          # 64
NS = N // 4         # 512 query rows per core
NCORES = 8
MT = N // 128       # 16 key tiles

_cache = {}


def _build_program(active, reps=1):
    import concourse.bass as bass
    import concourse.mybir as mybir
    import concourse.tile as tile
    from contextlib import ExitStack

    f32 = mybir.dt.float32
    f32r = mybir.dt.float32r
    bf16 = mybir.dt.bfloat16
    u8 = mybir.dt.uint8
    AFT = mybir.ActivationFunctionType
    ALU = mybir.AluOpType

    MTA = len(active)

    nc = bass.Bass()

    xtq = nc.declare_dram_parameter("xtq", [D, NS], bf16, isOutput=False)
    xtk = nc.declare_dram_parameter("xtk", [D, N], bf16, isOutput=False)
    xtv = nc.declare_dram_parameter("xtv", [D, N], bf16, isOutput=False)
    # E planes, partition-major so one DMA chunk is contiguous per partition:
    # [hp, p(=m%128), mi, j(head in pair), n]. bf16 (not u8): the all-16-bit
    # tensor_tensor E-multiply runs in the DVE 2x_1p fast mode.
    epl = nc.declare_dram_parameter("epl", [H // 2, 128, MTA, 2, NS], bf16,
                                    isOutput=False)
    wq = nc.declare_dram_parameter("wq", [D, D], bf16, isOutput=False)
    wk = nc.declare_dram_parameter("wk", [D, D], bf16, isOutput=False)
    wv = nc.declare_dram_parameter("wv", [D, D], bf16, isOutput=False)
    wo = nc.declare_dram_parameter("wo", [D, D], bf16, isOutput=False)
    bq2 = nc.declare_dram_parameter("bq2", [128, 4], f32, isOutput=False)
    bo2 = nc.declare_dram_parameter("bo2", [128, 4], f32, isOutput=False)
    # per-active-tile pad multiplier planes (zero padded key rows of V)
    pad = nc.declare_dram_parameter("pad", [128, MTA], f32, isOutput=False)
    pad8 = nc.declare_dram_parameter("pad8", [128, MTA, H], f32, isOutput=False)
    outT = nc.declare_dram_parameter("outT", [D, NS], f32, isOutput=True)

    with tile.TileContext(nc) as tc, ExitStack() as ctx:
        mm = nc.tensor.matmul
        _run_once(nc, tc, ctx, mm, tile, mybir, f32, f32r, bf16, u8,
                  AFT, ALU, active, xtq, xtk, xtv, epl, wq, wk, wv, wo,
                  bq2, bo2, pad, pad8, outT)

    _split_matmul_waits(nc, mybir)
    return nc


def _run_once(nc, tc, ctx, mm, tile, mybir, f32, f32r, bf16, u8, AFT, ALU,
              active, xtq, xtk, xtv, epl, wq, wk, wv, wo, bq2, bo2,
              pad, pad8, outT):
    from contextlib import ExitStack
    MTA = len(active)
    with ExitStack() as ctx:
        const_pool = ctx.enter_context(tc.tile_pool(name="const", bufs=1))
        persist = ctx.enter_context(tc.tile_pool(name="persist", bufs=1))

        # constants ride the Pool queue: SP's 650ns-per-DMA dispatch rate is
        # the lead-in bottleneck, so it is reserved for the q/k-path inputs
        loads = []
        bq_sb = const_pool.tile([128, 4], f32)
        loads.append(nc.gpsimd.dma_start(bq_sb[:], bq2[:]))
        bo_sb = const_pool.tile([128, 4], f32)
        loads.append(nc.gpsimd.dma_start(bo_sb[:], bo2[:]))
        pad_sb = const_pool.tile([128, MTA], f32)
        loads.append(nc.gpsimd.dma_start(pad_sb[:], pad[:]))
        pad8_sb = const_pool.tile([128, MTA, H], f32)
        loads.append(nc.gpsimd.dma_start(pad8_sb[:], pad8[:]))
        ones_sb = const_pool.tile([1, 64], f32)
        loads.append(nc.vector.memset(ones_sb[:], 1.0))
        wo_sb = persist.tile([128, 4, D], bf16)

        kT_sb = persist.tile([128, 4, N], bf16)
        qT_sb = persist.tile([128, 4, NS], bf16)
        v_sb = persist.tile([128, MTA, H, C + 1], bf16)
        OT_sb = persist.tile([128, 4, NS], bf16)
        outT_sb = persist.tile([128, 4, NS], f32)

        # ---- Phase A (part 1): DMAs + projections needed by pass 0 ----
        # Emission order software-pipelines the phases: q/k projections for
        # heads 0-3 (j-blocks 0,1) and all of v before pass 0; j-blocks 2,3
        # and the wo load are deferred into pass 0's normalize window.
        xw_pool = ctx.enter_context(tc.tile_pool(name="xw", bufs=1))
        wq_sb = xw_pool.tile([128, 4, D], bf16, tag="w")
        wk_sb = xw_pool.tile([128, 4, D], bf16, tag="w2")
        wv_sb = xw_pool.tile([128, 4, D], bf16, tag="w3")
        xtq_sb = xw_pool.tile([128, 4, NS], bf16, tag="xq")
        xtk_sb = xw_pool.tile([128, 4, N], bf16, tag="xk")
        xtv_sb = xw_pool.tile([128, 4, N], bf16, tag="xv")
        # q/k path on SP, v path on the Pool queue: parallel dispatch halves
        # the dispatch-bound lead-in. One consolidated DMA per weight matrix;
        # x_k / x_v arrive in column chunks so the first projection blocks
        # start before the full 2MB lands.
        nc.sync.dma_start(wq_sb[:], wq[:].rearrange("(c p) d -> p c d", p=128))
        nc.sync.dma_start(xtq_sb[:],
                          xtq[:].rearrange("(c p) n -> p c n", p=128))
        nc.sync.dma_start(wk_sb[:], wk[:].rearrange("(c p) d -> p c d", p=128))
        nc.gpsimd.dma_start(wv_sb[:],
                            wv[:].rearrange("(c p) d -> p c d", p=128))
        for mb in range(4):
            sl = slice(mb * 512, (mb + 1) * 512)
            nc.sync.dma_start(
                xtk_sb[:, :, sl],
                xtk[:, sl].rearrange("(kc p) m -> p kc m", p=128))
            nc.gpsimd.dma_start(
                xtv_sb[:, :, sl],
                xtv[:, sl].rearrange("(kc p) m -> p kc m", p=128))

        # E-plane tiles: quarter planes [128, 4, 2, NS] bf16 per fetch; the
        # first two ride the SP queue right after the input DMAs (so they
        # don't preempt them on the DMA engines), the rest go on the Pool
        # queue spread through the passes
        ep_pool = ctx.enter_context(tc.tile_pool(name="ep", bufs=_B("EP", 6)))
        pT_pool = ctx.enter_context(tc.tile_pool(name="pT", bufs=_B("PT", 5)))
        p2_pool = ctx.enter_context(tc.tile_pool(name="p2", bufs=_B("P2", 5)))
        small_pool = ctx.enter_context(tc.tile_pool(name="small", bufs=8))
        EH = 4
        nequarters = (MTA + EH - 1) // EH
        ep_tiles = {}

        def fetch_e(hp, q, eng=None):
            h0 = q * EH
            hn = min(EH, MTA - h0)
            t = ep_pool.tile([128, EH, 2, NS], bf16, tag="ep")
            (eng or nc.gpsimd).dma_start(t[:, 0:hn, :, :],
                                         epl[hp, :, h0:h0 + hn, :, :])
            ep_tiles[(hp, q)] = t

        # phase-A E fetches ride the Pool queue AFTER the v-path inputs: the
        # DMA device serves queue heads first-come, so this ordering keeps
        # the projection inputs ahead of the (less urgent) E planes. ACT is
        # unusable for this: its DMA dispatch costs ~4.4us per copy and
        # would stall the first evacuation activations.
        fetch_e(0, 0)
        fetch_e(1, 0)
        fetch_e(0, 1)
        fetch_e(1, 1)
        fetch_e(0, 2)
        fetch_e(1, 2)

        vones = [nc.vector.tensor_copy(
            v_sb[:, :, :, C : C + 1],
            pad8_sb[:, :, :].rearrange("p m (h o) -> p m h o", o=1))]

        with tc.tile_pool(name="psA", bufs=_B("PSA", 8), space="PSUM") as psA:
            # qT[dh, n] = (Wq/8).T @ xT_q  (+ bq/8 per-partition), heads 0-3
            for j in range(2):
                ps = psA.tile([128, NS], f32, tag="psA")
                for kc in range(4):
                    mm(ps[:], wq_sb[:, kc, j * 128:(j + 1) * 128],
                       xtq_sb[:, kc, :], start=(kc == 0), stop=(kc == 3))
                nc.scalar.activation(qT_sb[:, j, :], ps[:], AFT.Identity,
                                     bias=bq_sb[:, j:j + 1])

            # kT[dh, m] = Wk.T @ xT_k, heads 0-3 (k bias drops in softmax)
            for mb in range(4):
                for j in range(2):
                    ps = psA.tile([128, NS], f32, tag="psA")
                    for kc in range(4):
                        mm(ps[:], wk_sb[:, kc, j * 128:(j + 1) * 128],
                           xtk_sb[:, kc, mb * 512:(mb + 1) * 512],
                           start=(kc == 0), stop=(kc == 3))
                    nc.scalar.copy(kT_sb[:, j, mb * 512:(mb + 1) * 512], ps[:])

            # v[m, c] = xT_v.T @ Wv, padded key rows zeroed (scale by pad01)
            for mi, mt in enumerate(active):
                ps = psA.tile([128, D], f32, tag="psA")
                for kc in range(4):
                    mm(ps[:], xtv_sb[:, kc, mt * 128:(mt + 1) * 128],
                       wv_sb[:, kc, :], start=(kc == 0), stop=(kc == 3))
                nc.vector.tensor_scalar(
                    v_sb[:, mi, :, 0:C],
                    ps[:].rearrange("p (h c) -> p h c", h=H),
                    pad_sb[:, mi:mi + 1], None, ALU.mult)

        # PSUM pools for phase B (psA released its banks above)
        # psS tiles are [128, 2, NS] (2 banks): two heads' scores per tile so
        # one Exp instruction covers 1024 elements. 2 bufs (4 banks) + psO 4.
        psO = ctx.enter_context(tc.tile_pool(name="psO", bufs=4, space="PSUM"))
        psS = ctx.enter_context(tc.tile_pool(name="psS", bufs=_B("PSS", 2), space="PSUM"))

        # ---- Phase B: attention, two passes of 4 heads (2 head pairs) ----
        def attn_pass(p):
            o_ps = [psO.tile([128, NS], f32, tag="psO", name=f"o_ps{p}_{i}")
                    for i in range(4)]
            for mi in range(MTA):
                for hpl in range(2):
                    hp = 2 * p + hpl
                    if p == 0 and hpl == 0 and mi in (6, 8, 10, 12):
                        nq = {6: [(0, 3), (1, 3)], 8: [(2, 0), (3, 0)],
                              10: [(2, 1), (3, 1)], 12: [(2, 2), (3, 2)]}[mi]
                        for a, b in nq:
                            fetch_e(a, b)
                    if p == 1 and hpl == 0 and mi == 4:
                        fetch_e(2, 3)
                        fetch_e(3, 3)
                    s_ps = psS.tile([128, 2, NS], f32, tag="psS")
                    for j in range(2):
                        h = 4 * p + 2 * hpl + j
                        hj, ho = h // 2, (h % 2) * 64
                        mm(s_ps[:, j, :],
                           kT_sb[ho:ho + 64, hj, active[mi] * 128:active[mi] * 128 + 128],
                           qT_sb[ho:ho + 64, hj, :], start=True, stop=True)
                    pT = pT_pool.tile([128, 2, NS], bf16, tag="pT")
                    nc.scalar.activation(pT[:], s_ps[:], AFT.Exp)
                    p2 = p2_pool.tile([128, 2, NS], bf16, tag="p2")
                    # all-bf16 tensor_tensor: DVE 2x_1p fast mode (~593ns)
                    nc.vector.tensor_tensor(
                        p2[:], pT[:],
                        ep_tiles[(hp, mi // EH)][:, mi % EH, :, :], ALU.mult)
                    for j in range(2):
                        h = 4 * p + 2 * hpl + j
                        mm(o_ps[2 * hpl + j][0:65, :], v_sb[:, mi, h, :],
                           p2[:, j, :],
                           start=(mi == 0), stop=(mi == MTA - 1))
            return o_ps

        def normalize(p, o_ps, halves=(0, 1)):
            # OT[h-rows, n] = o[c, n] / rowsum[n]. All reciprocal broadcasts
            # of this call pack into ONE psS tile (two per bank on opposite
            # partition halves) and are read straight from PSUM, so the psS
            # ring turns over once instead of four times per pass.
            bps = psS.tile([128, 2, NS], f32, tag="psS", name=f"bps{p}")
            for hx, half in enumerate(halves):
                idx = [2 * half, 2 * half + 1]
                rsbs = []
                for i in idx:
                    rsb = small_pool.tile([1, NS], f32, tag="rsb",
                                          name=f"rsb{p}_{i}")
                    nc.vector.reciprocal(rsb[:], o_ps[i][64:65, :])
                    rsbs.append(rsb)
                for k, i in enumerate(idx):
                    mm(bps[64 * k:64 * k + 64, hx, :], ones_sb[0:1, :],
                       rsbs[k][0:1, :], start=True, stop=True)
                b_sb = small_pool.tile([128, NS], f32, tag="bsb",
                                       name=f"bsb{p}_{half}")
                nc.vector.tensor_copy(b_sb[:], bps[:, hx, :])
                for k, i in enumerate(idx):
                    h = 4 * p + i
                    hj, ho = h // 2, (h % 2) * 64
                    nc.vector.tensor_tensor(
                        OT_sb[ho:ho + 64, hj, :], o_ps[i][0:64, :],
                        b_sb[64 * k:64 * k + 64, :], ALU.mult)

        # deferred projections for heads 4-7 (j-blocks 2,3): the first k block
        # goes through the psS ring right at pass-0 end; the rest run in psO
        # slots as pass 0's normalize halves release them, keeping the psS
        # ring free so pass 1's score pipeline restarts immediately
        def proj_j23_k_psS(mb):
            ps = psS.tile([128, 2, NS], f32, tag="psS", name=f"kp{mb}")
            for j in (2, 3):
                for kc in range(4):
                    mm(ps[:, j - 2, :], wk_sb[:, kc, j * 128:(j + 1) * 128],
                       xtk_sb[:, kc, mb * 512:(mb + 1) * 512],
                       start=(kc == 0), stop=(kc == 3))
            # evacuations split ACT/DVE: at the pass boundary DVE serializes
            # the normalize chain, while ACT is idle
            nc.scalar.copy(kT_sb[:, 2, mb * 512:(mb + 1) * 512], ps[:, 0, :])
            nc.vector.tensor_copy(
                kT_sb[:, 3, mb * 512:(mb + 1) * 512], ps[:, 1, :])

        def proj_j23_q_psS():
            ps = psS.tile([128, 2, NS], f32, tag="psS", name="qp23")
            for j in (2, 3):
                for kc in range(4):
                    mm(ps[:, j - 2, :], wq_sb[:, kc, j * 128:(j + 1) * 128],
                       xtq_sb[:, kc, :], start=(kc == 0), stop=(kc == 3))
            for j in (2, 3):
                nc.scalar.activation(qT_sb[:, j, :], ps[:, j - 2, :],
                                     AFT.Identity, bias=bq_sb[:, j:j + 1])

        def proj_j23_k_psO(mb):
            for j in (2, 3):
                ps = psO.tile([128, NS], f32, tag="psO", name=f"kp{mb}_{j}")
                for kc in range(4):
                    mm(ps[:], wk_sb[:, kc, j * 128:(j + 1) * 128],
                       xtk_sb[:, kc, mb * 512:(mb + 1) * 512],
                       start=(kc == 0), stop=(kc == 3))
                if j == 2:
                    nc.scalar.copy(
                        kT_sb[:, j, mb * 512:(mb + 1) * 512], ps[:])
                else:
                    nc.vector.tensor_copy(
                        kT_sb[:, j, mb * 512:(mb + 1) * 512], ps[:])

        o_ps0 = attn_pass(0)
        proj_j23_k_psS(0)
        proj_j23_q_psS()
        normalize(0, o_ps0, (0,))
        proj_j23_k_psO(1)
        normalize(0, o_ps0, (1,))
        proj_j23_k_psO(2)
        proj_j23_k_psO(3)
        for c in range(4):
            nc.sync.dma_start(wo_sb[:, c, :], wo[c * 128:(c + 1) * 128, :])

        o_ps1 = attn_pass(1)

        # ---- Phase C: output projection, pipelined with pass-1 normalize.
        # After half 0 (heads 4,5 -> OT[:,2]) two psO slots free: start the
        # g=0..2 partial accumulation for the first two output blocks; the
        # g=3 term and the remaining blocks follow half 1.
        def outproj_partial(jt):
            ps = psO.tile([128, NS], f32, tag="psO", name=f"oc{jt}")
            for g in (0, 1, 2):
                mm(ps[:], wo_sb[:, g, jt * 128:(jt + 1) * 128],
                   OT_sb[:, g, :], start=(g == 0), stop=False)
            return ps

        def outproj_finish(jt, ps=None):
            if ps is None:
                ps = psO.tile([128, NS], f32, tag="psO", name=f"oc{jt}")
                for g in (0, 1, 2):
                    mm(ps[:], wo_sb[:, g, jt * 128:(jt + 1) * 128],
                       OT_sb[:, g, :], start=(g == 0), stop=False)
            mm(ps[:], wo_sb[:, 3, jt * 128:(jt + 1) * 128],
               OT_sb[:, 3, :], start=False, stop=True)
            nc.scalar.activation(outT_sb[:, jt, :], ps[:], AFT.Identity,
                                 bias=bo_sb[:, jt:jt + 1])
            nc.sync.dma_start(outT[jt * 128:(jt + 1) * 128, :],
                              outT_sb[:, jt, :])

        normalize(1, o_ps1, (0,))
        pc0 = outproj_partial(0)
        pc1 = outproj_partial(1)
        normalize(1, o_ps1, (1,))
        outproj_finish(0, pc0)
        outproj_finish(1, pc1)
        outproj_finish(2)
        outproj_finish(3)


# every TPB instruction encoding in this walrus build tolerates only a
# single semaphore wait -- split extras regardless of opcode
_NO_SPLIT_TYPES = {"InstEventSemaphore"}


def _split_matmul_waits(nc, mybir):
    """Several engine instruction encodings tolerate only one semaphore
    wait; move extra waits onto standalone single-wait EventSemaphore
    instructions inserted right before them on the same engine queue."""
    import bass_rust

    n = 0
    for bb in nc.m.functions[0].blocks:
        insts = list(bb.instructions)
        out = []
        changed = False
        for i in insts:
            si = i.sync_info
            if (type(i).__name__ not in _NO_SPLIT_TYPES and si is not None
                    and len(si.on_wait) > 1):
                w = list(si.on_wait)
                for wx in w[:-1]:
                    ev = mybir.InstEventSemaphore(name=f"mmw_{n}_{i.name}",
                                                  ins=[], outs=[])
                    ev.engine = i.engine
                    ev.sync_info = bass_rust.SyncInfo(on_wait=[wx],
                                                      on_update=[])
                    out.append(ev)
                    n += 1
                si.on_wait = [w[-1]]
                changed = True
            out.append(i)
        if changed:
            bb.instructions = out


def _host_prep(inputs):
    x_q = np.asarray(inputs["x_q"], np.float32)
    x_k = np.asarray(inputs["x_k"], np.float32)
    x_v = np.asarray(inputs["x_v"], np.float32)
    attn_mask = np.asarray(inputs["attn_mask"]).astype(bool)
    kpm = np.asarray(inputs["key_padding_mask"]).astype(bool)
    Wq = np.asarray(inputs["Wq"], np.float32)
    Wk = np.asarray(inputs["Wk"], np.float32)
    Wv = np.asarray(inputs["Wv"], np.float32)
    Wo = np.asarray(inputs["Wo"], np.float32)
    bq = np.asarray(inputs["bq"], np.float32)
    bv = np.asarray(inputs["bv"], np.float32)
    bo = np.asarray(inputs["bo"], np.float32)
    mw = np.asarray(inputs["mask_weight"], np.float64)

    # c[h,r] = softmax(mask_weight[h,:R]) * mask_weight[h,R]
    e = np.exp(mw[:, :R] - mw[:, :R].max(axis=1, keepdims=True))
    w = e / e.sum(axis=1, keepdims=True)
    c = (w * mw[:, R:R + 1]).astype(np.float32)          # [H, R]

    # active key tiles (at least one unpadded key) -- shared across batch
    # so a single compiled program serves all cores
    tile_padded = kpm.reshape(B, MT, 128).all(axis=2)    # [B, MT]
    active = [mt for mt in range(MT) if not tile_padded[:, mt].all()]
    MTA = len(active)

    scale = np.float32(1.0 / np.sqrt(C))
    wq_s = (Wq * scale).astype(np.float32)
    bq_s = (bq * scale).astype(np.float32)
    bo_p = (bv @ Wo + bo).astype(np.float32)

    bq2 = np.ascontiguousarray(bq_s.reshape(4, 128).T)
    bo2 = np.ascontiguousarray(bo_p.reshape(4, 128).T)

    bf = ml_dtypes.bfloat16
    common = dict(wq=wq_s.astype(bf), wk=Wk.astype(bf), wv=Wv.astype(bf),
                  wo=Wo.astype(bf), bq2=bq2, bo2=bo2)

    # E planes per (b, n-slice): E[h, mi, p, n] = round(255*exp(-bias))
    emul = np.exp(-c)                                    # [H, R] in (0,1]
    in_maps = []
    for core in range(NCORES):
        b, ns = core // 4, core % 4
        n0 = ns * NS
        pad01 = (~kpm[b]).astype(np.float32)             # [N]
        pad2 = np.ascontiguousarray(
            pad01.reshape(MT, 128).T[:, active])         # [128, MTA]
        pad8 = np.ascontiguousarray(np.repeat(pad2[:, :, None], H, axis=2))
        # reference adds rel = +sum_r c[h,r]*(~attn_mask); shifting by the
        # softmax-invariant constant sum_r c[h,r] gives the bounded form
        # E = exp(-sum_r c[h,r]*attn_mask) in (0,1], ideal for u8
        inv = attn_mask[b, :, n0:n0 + NS, :]             # [R, NS, N]
        # compute per active tile to keep memory small, u8 quantized
        # partition-major: [hp, p(=m within tile), mi, j(head in pair), n]
        ep = np.empty((H // 2, 128, MTA, 2, NS), bf)
        for mi, mt in enumerate(active):
            invt = inv[:, :, mt * 128:(mt + 1) * 128]    # [R, NS, 128]
            bias = np.einsum('hr,rnm->hmn', c, invt.astype(np.float32))
            ep[:, :, mi] = np.exp(-bias).astype(bf).reshape(
                H // 2, 2, 128, NS).transpose(0, 2, 1, 3)
        ep = np.ascontiguousarray(ep)
        m = dict(common)
        m["xtq"] = np.ascontiguousarray(x_q[n0:n0 + NS, b, :].T).astype(bf)
        m["xtk"] = np.ascontiguousarray(x_k[:, b, :].T).astype(bf)
        m["xtv"] = np.ascontiguousarray(x_v[:, b, :].T).astype(bf)
        m["epl"] = ep
        m["pad"] = pad2
        m["pad8"] = pad8
        in_maps.append(m)
    return in_maps, active


def kernel(**inputs) -> np.ndarray:
    from concourse.bass_utils import run_bass_kernel_spmd

    in_maps, active = _host_prep(inputs)
    key = tuple(active)
    if key not in _cache:
        _cache[key] = _build_program(active)
        _cache["nc"] = _cache[key]
    nc = _cache[key]

    res = run_bass_kernel_spmd(nc, in_maps, list(range(NCORES)))

    out = np.empty((N, B, D), np.float32)
    for core in range(NCORES):
        b, ns = core // 4, core % 4
        n0 = ns * NS
        out[n0:n0 + NS, b, :] = res.results[core]["outT"].T
    return out
